# revision 24
# baseline (speedup 1.0000x reference)
"""Trainium2 Bass kernel for nn_BasicRNN (2-layer LSTM, H=32, S=64, B=8192).

Strategy: pure data parallel over 8 cores (1024 batch each). Per core the
batch is laid out in "T-layout" tiles [128 partitions = 4 groups x 32
features, 256 batch (free)]. The 256-batch free dim is split into two
128-wide chunks (A/B) whose dependency chains software-pipeline across the
engines.

Per chunk-step the gate pre-activations build up in one PSUM bank
[128, 512] = (f|i|o|g) x 128 batch:
  - biases folded into the matmuls (layer 1: extra ones-row in the DMA'd
    x data; layer 2: one K=4 "bias matmul" against a constant block-ones
    rhs),
  - input projections as 4 block-diagonal fp16 matmuls (M=128, start
    group),
  - recurrent h-matmuls as 4 block-diagonal fp16 matmuls (M=128,
    accumulate, stop).
Then ONE scalar-engine tanh over all 4 gates at once ([128,512] PSUM ->
fp16 SBUF), the doubled-cell update as fp16 DVE scalar_tensor_tensor ops,
one small tanh(c*/2), and the h update. Sigmoids use the tanh identity
sigmoid(x) = (1 + tanh(x/2))/2 with all scale factors folded into the
host-prepped weights; cell and hidden state are stored doubled (c* = 2c,
h* = 2h):
    u  = (tanh_f + 1) * c*          v = (tanh_i + 1) * tanh_g
    c* = 0.5*u + v
    h* = (tanh_o + 1) * tanh(0.5 * c*)
"""
import sys
sys.path.insert(0, '/opt/trn_rl_repo')

import numpy as np

import concourse.bacc as bacc
import concourse.tile as tile
from concourse import mybir
from concourse.bass_utils import run_bass_kernel_spmd

F32 = mybir.dt.float32
F16 = mybir.dt.float16
TANH = mybir.ActivationFunctionType.Tanh
IDENT = mybir.ActivationFunctionType.Identity
ADD = mybir.AluOpType.add
MULT = mybir.AluOpType.mult

B, S, NX, NSFC, H, NY = 8192, 64, 4, 5, 32, 1
NCORES = 8
BC = B // NCORES          # 1024 batch per core
NG = 4                    # groups per core (partition strips)
GB = BC // NG             # 256 batch per group
CB = GB // 2              # 128 batch per chunk (A/B split of the free dim)
# gate order in the G-tile free dim: (g, f, i, o), 128 cols each.
GATES = [("g", 2 * H, 1.0), ("f", H, 0.5), ("i", 0, 0.5), ("o", 3 * H, 0.5)]

_CACHED = {}


def _prep_weights(inp):
    """Host-side weight staging. Returns dict of DRAM arrays (shared by all
    cores)."""
    w = {}

    def blockdiag(wmat, scale_fio, scale_g):
        # [128, 512]: col block gi = block-diag lhsT for gate gi.
        t = np.zeros((128, 512), np.float32)
        for gi, (nm, r0, _) in enumerate(GATES):
            s = scale_g if nm == "g" else scale_fio
            blk = (wmat[r0:r0 + H] * s).T.astype(np.float32)  # [K=H, 32]
            for g in range(NG):
                t[32 * g:32 * g + H,
                  128 * gi + 32 * g:128 * gi + 32 * g + 32] = blk
        return t.astype(np.float16)

    # layer-1 input lhsT [20, 512]: rows 5g+q (q<4 -> x features, q=4 ->
    # bias row). x is true scale; output h1* doubled elsewhere.
    t = np.zeros((20, 512), np.float32)
    btot1 = inp["b_ih1"] + inp["b_hh1"]
    for gi, (nm, r0, trick) in enumerate(GATES):
        s = 1.0 if nm == "g" else 0.5
        blk = (inp["w_ih1"][r0:r0 + H] * s).T.astype(np.float32)  # [NX, 32]
        for g in range(NG):
            c0 = 128 * gi + 32 * g
            t[5 * g:5 * g + NX, c0:c0 + 32] = blk
            t[5 * g + NX, c0:c0 + 32] = btot1[r0:r0 + H] * trick
    w["WX1"] = t.astype(np.float16)

    w["WL1"] = blockdiag(inp["w_hh1"], 0.25, 0.5)
    w["WX2"] = blockdiag(inp["w_ih2"], 0.25, 0.5)   # input h1* is doubled
    w["WL2"] = blockdiag(inp["w_hh2"], 0.25, 0.5)

    # layer-2 bias matmul: lhsT B2 [128, 128]: row 32g+a holds gate-a bias
    # values at cols 32g+j; rhs ONESB [128, 512]: row 32g+a is 1.0 exactly
    # on gate-a's 128-col block.
    b2 = np.zeros((128, 128), np.float32)
    onesb = np.zeros((128, 512), np.float32)
    btot2 = inp["b_ih2"] + inp["b_hh2"]
    for gi, (_, r0, trick) in enumerate(GATES):
        for g in range(NG):
            b2[32 * g + gi, 32 * g:32 * g + 32] = btot2[r0:r0 + H] * trick
            onesb[32 * g + gi, 128 * gi:128 * (gi + 1)] = 1.0
    w["B2"] = b2.astype(np.float16)
    w["ONESB"] = onesb.astype(np.float16)

    # sfc weights [8, 64]: rows 0:5 = [w_sfc1.T | w_sfc2.T]
    ws = np.zeros((8, 64), np.float32)
    ws[:NSFC, 0:32] = inp["w_sfc1"].T
    ws[:NSFC, 32:64] = inp["w_sfc2"].T
    w["WSFC"] = ws
    bs = np.zeros((128, 2), np.float32)
    for g in range(NG):
        bs[32 * g:32 * g + 32, 0] = inp["b_sfc1"]
        bs[32 * g:32 * g + 32, 1] = inp["b_sfc2"]
    w["BSFC"] = bs

    # output weights [128, 1]: block g = (w_out * 0.5).T
    wo = np.zeros((128, 1), np.float32)
    for g in range(NG):
        wo[32 * g:32 * g + 32, 0] = inp["w_out"][0] * 0.5
    w["WOUT"] = wo.astype(np.float16)
    w["BOUT"] = np.full((128, 1), float(inp["b_out"][0]), np.float32)
    return w


def build_program(n_steps=S, trace_sim=False, reps=0):
    nc = bacc.Bacc()
    d = {}
    d["xs"] = nc.declare_dram_parameter("xs", [n_steps, 20, GB], F16,
                                        isOutput=False)
    d["sfcT"] = nc.declare_dram_parameter("sfcT", [8, BC], F32, isOutput=False)
    F16W = {"WX1", "WL1", "WX2", "WL2", "B2", "ONESB", "WOUT"}
    WSHAPES = [("WX1", [20, 512]), ("WL1", [128, 512]),
               ("WX2", [128, 512]), ("WL2", [128, 512]),
               ("B2", [128, 128]), ("ONESB", [128, 512]),
               ("WSFC", [8, 64]), ("BSFC", [128, 2]),
               ("WOUT", [128, 1]), ("BOUT", [128, 1])]
    for nm, shape in WSHAPES:
        d[nm] = nc.declare_dram_parameter(nm, shape,
                                          F16 if nm in F16W else F32,
                                          isOutput=False)
    y_out = nc.declare_dram_parameter("y", [NG, n_steps * GB], F32,
                                      isOutput=True)

    NS1 = n_steps + 1

    with tile.TileContext(nc, trace_sim=trace_sim) as tc:
        with tc.tile_pool(name="wpool", bufs=1) as wpool, \
             tc.tile_pool(name="big", bufs=1) as big, \
             tc.tile_pool(name="state", bufs=1) as state, \
             tc.tile_pool(name="work", bufs=3) as work, \
             tc.tile_pool(name="xp", bufs=3) as xp, \
             tc.tile_pool(name="yp", bufs=2) as yp, \
             tc.tile_pool(name="psA", bufs=2, space="PSUM") as psA, \
             tc.tile_pool(name="psY", bufs=1, space="PSUM") as psY:

            # ---- stage weights ----
            W = {}
            for nm, shape in WSHAPES:
                t = wpool.tile(shape, F16 if nm in F16W else F32, tag=nm)
                nc.sync.dma_start(t[:], d[nm][:])
                W[nm] = t
            sfcT = wpool.tile([8, BC], F32, tag="sfcT")
            nc.sync.dma_start(sfcT[:], d["sfcT"][:])

            # ---- big state storage ----
            h1_all = big.tile([128, NS1 * GB], F16, tag="h1_all")
            h2_all = big.tile([128, NS1 * GB], F16, tag="h2_all")

            # CT tiles [128, 5*CB] fp16: [c*slot | tg | tf | ti | to]
            def new_CT(ch):
                return work.tile([128, 5 * CB], F16, tag=f"CT{ch}",
                                 name=f"CT{ch}")

            # ---- init: h0/c0 from surface MLPs ----
            ph = psA.tile([128, 2 * GB], F32, tag="G0")
            for g in range(NG):
                nc.tensor.matmul(ph[32 * g:32 * g + 32, 0:GB],
                                 W["WSFC"][0:NSFC, 0:32],
                                 sfcT[0:NSFC, GB * g:GB * (g + 1)],
                                 start=True, stop=True,
                                 tile_position=(0, 32 * g))
                nc.tensor.matmul(ph[32 * g:32 * g + 32, GB:2 * GB],
                                 W["WSFC"][0:NSFC, 32:64],
                                 sfcT[0:NSFC, GB * g:GB * (g + 1)],
                                 start=True, stop=True,
                                 tile_position=(0, 32 * g))
            t0 = work.tile([128, GB], F32, tag="t0")
            nc.scalar.activation(t0[:], ph[:, 0:GB], TANH, bias=W["BSFC"][:, 0:1])
            # h0* = 2*tanh(...)  stored at h1_all slice n_steps
            nc.vector.tensor_scalar_mul(
                h1_all[:, n_steps * GB:(n_steps + 1) * GB], t0[:], 2.0)
            t0b = work.tile([128, GB], F32, tag="t0")
            nc.scalar.activation(t0b[:], ph[:, GB:2 * GB], TANH,
                                 bias=W["BSFC"][:, 1:2])
            # layer-2 zero init state
            nc.vector.memset(h2_all[:, 0:GB], 0.0)

            # ---- the two sequential LSTM layers ----
            def emit_inputs(layer, k, ch, xstep):
                """Start a fresh PSUM gates tile for (k, ch): bias + input
                projections (start group, no stop)."""
                G = psA.tile([128, 4 * CB], F32, tag=f"G{ch}")
                if layer == 1:
                    for gi in range(4):
                        nc.tensor.matmul(
                            G[:, gi * CB:(gi + 1) * CB],
                            W["WX1"][0:20, gi * 128:(gi + 1) * 128],
                            xstep[0:20, ch * CB:(ch + 1) * CB],
                            start=(gi == 0), stop=False)
                else:
                    nc.tensor.matmul(G[:, 0:4 * CB], W["B2"][:, 0:128],
                                     W["ONESB"][:, 0:4 * CB],
                                     start=True, stop=False)
                    for gi in range(4):
                        nc.tensor.matmul(
                            G[:, gi * CB:(gi + 1) * CB],
                            W["WX2"][:, gi * 128:(gi + 1) * 128],
                            h1_all[:, k * GB + ch * CB:k * GB + (ch + 1) * CB],
                            start=False, stop=False)
                return G

            def scan_body(iv=None):
                for layer in (1, 2):
                    WL = W["WL1"] if layer == 1 else W["WL2"]
                    hall = h1_all if layer == 1 else h2_all

                    # prologue: first xstep DMA + step-0 input projections
                    if layer == 1:
                        xstep = xp.tile([20, GB], F16, tag="x")
                        nc.sync.dma_start(xstep[:], d["xs"][0])
                    else:
                        xstep = None
                    Tcur = []
                    for ch in (0, 1):
                        t = new_CT(ch)
                        if layer == 1:
                            nc.vector.tensor_scalar_mul(
                                t[:, 0:CB],
                                t0b[:, ch * CB:(ch + 1) * CB], 2.0)
                        else:
                            nc.vector.memset(t[:, 0:CB], 0.0)
                        Tcur.append(t)
                    Gcur = [emit_inputs(layer, 0, ch, xstep) for ch in (0, 1)]

                    for k in range(n_steps):
                        if layer == 1:
                            rhs_idx, out_idx = n_steps - k, n_steps - 1 - k
                        else:
                            rhs_idx, out_idx = k, k + 1
                        if layer == 1 and k + 1 < n_steps:
                            xnext = xp.tile([20, GB], F16, tag="x")
                            nc.sync.dma_start(xnext[:], d["xs"][k + 1])
                        else:
                            xnext = None

                        for ch in (0, 1):
                            G = Gcur[ch]
                            rhs = hall[:, rhs_idx * GB + ch * CB:
                                       rhs_idx * GB + (ch + 1) * CB]
                            for gi in range(4):
                                nc.tensor.matmul(
                                    G[:, gi * CB:(gi + 1) * CB],
                                    WL[:, gi * 128:(gi + 1) * 128],
                                    rhs, start=False, stop=(gi == 3))
                            CT = Tcur[ch]
                            nc.scalar.activation(CT[:, CB:5 * CB], G[:], TANH)
                            CTn = new_CT(ch)
                            UV = work.tile([128, 2 * CB], F16, tag="UV")
                            # u = (tf+1)*c*,  v = (ti+1)*tg (fused)
                            nc.vector.scalar_tensor_tensor(
                                UV[:], CT[:, 2 * CB:4 * CB], 1.0,
                                CT[:, 0:2 * CB], ADD, MULT)
                            # c* = 0.5*u + v -> next CT's c-slot
                            nc.vector.scalar_tensor_tensor(
                                CTn[:, 0:CB], UV[:, 0:CB], 0.5,
                                UV[:, CB:2 * CB], MULT, ADD)
                            TC = work.tile([128, CB], F16, tag="TC")
                            nc.scalar.activation(TC[:], CTn[:, 0:CB],
                                                 TANH, scale=0.5)
                            # h* = (to+1)*tanh(c)
                            nc.vector.scalar_tensor_tensor(
                                hall[:, out_idx * GB + ch * CB:
                                     out_idx * GB + (ch + 1) * CB],
                                CT[:, 4 * CB:5 * CB], 1.0, TC[:], ADD, MULT)
                            Tcur[ch] = CTn
                            if k + 1 < n_steps:
                                Gcur[ch] = emit_inputs(layer, k + 1, ch, xnext)

            if reps:
                with tc.For_i(0, reps, 1) as iv:
                    scan_body(iv)
            else:
                scan_body()

            # ---- output projection: y = h2* @ (w_out/2).T + b_out ----
            YCH = 1024                       # free elems per chunk
            total = n_steps * GB
            nch = total // YCH
            for ci in range(nch):
                py = psY.tile([128, YCH], F32, tag="PY")
                for g in range(NG):
                    for j in range(YCH // 512):
                        off = GB + ci * YCH + j * 512
                        nc.tensor.matmul(py[32 * g:32 * g + 1,
                                            j * 512:(j + 1) * 512],
                                         W["WOUT"][32 * g:32 * g + 32, 0:1],
                                         h2_all[32 * g:32 * g + 32,
                                                off:off + 512],
                                         start=True, stop=True,
                                         tile_position=(32 * g, 32 * g))
                ysb = yp.tile([128, YCH], F32, tag="ysb")
                nc.scalar.activation(ysb[:], py[:], IDENT, bias=W["BOUT"][:, 0:1])
                for g in range(NG):
                    nc.sync.dma_start(y_out[g, ci * YCH:(ci + 1) * YCH],
                                      ysb[32 * g:32 * g + 1, :])
    nc.finalize()
    return nc


def kernel(**inputs):
    inputs = {k: np.asarray(v) for k, v in inputs.items()}
    if "nc" not in _CACHED:
        _CACHED["nc"] = build_program(S)
    nc = _CACHED["nc"]

    wts = _prep_weights(inputs)
    x = inputs["inputs_main"]          # [B, S, NX]
    sfc = inputs["inputs_sfc"]         # [B, NSFC]

    in_maps = []
    for c in range(NCORES):
        xs_c = x[c * BC:(c + 1) * BC]          # [BC, S, NX]
        sfc_c = sfc[c * BC:(c + 1) * BC]       # [BC, NSFC]
        # xs[s, 5g+q, r] = x[256g+r, S-1-s, q] for q<4; 1.0 for q=4
        xr = xs_c[:, ::-1, :]                  # time reversed
        xg = xr.reshape(NG, GB, S, NX).transpose(2, 0, 3, 1)  # [S, NG, NX, GB]
        xs_arr = np.ones((S, NG, 5, GB), np.float32)
        xs_arr[:, :, :NX, :] = xg
        xs_arr = xs_arr.reshape(S, 20, GB)
        sfcT = np.zeros((8, BC), np.float32)
        sfcT[:NSFC] = sfc_c.T
        m = {"xs": xs_arr.astype(np.float16), "sfcT": sfcT}
        m.update(wts)
        in_maps.append(m)

    res = run_bass_kernel_spmd(nc, in_maps, list(range(NCORES)))

    y = np.empty((B, S, NY), np.float32)
    for c in range(NCORES):
        yc = res.results[c]["y"]               # [NG, S*GB]
        yc = yc.reshape(NG, S, GB).transpose(0, 2, 1)   # [NG, GB, S]
        y[c * BC:(c + 1) * BC, :, 0] = yc.reshape(BC, S)
    return y
